# revision 1
# baseline (speedup 1.0000x reference)
"""Trainium2 Bass kernel for nn_AHardPair (hard-pair mining loss).

Self-contained: computes loss = mean(a_lr * (pos_loss + neg_loss)) over
8192 L2-normalized embeddings of dim 128, classes = contiguous blocks
of 8 rows.  The 8192x8192 pairwise-distance work is sharded over 8
NeuronCores; each distance pair is computed once (symmetric circular
column slabs; slots 0-3 carry 33 blocks including the delta=32 block,
slots 4-7 carry 32), with per-core column rotation so all cores run one
SPMD program.

Device per 128-row slot (8 slots/core):
  G = x_rows . x_cols^T (bf16 TensorE matmuls, diag block poisoned)
  d = sqrt(2 - 2G)  (ScalarE, PSUM->SBUF bf16, table preloaded)
  e1 = exp(40-40d), e2 = exp(22-20d)  (ScalarE, free row-sum accum)
  column sums of e over the slab (ones-matmul -> PSUM -> DVE/ACT -> DRAM)
  d diag blocks shipped for host-side same-class stats.
Host: scatter colsums (symmetric cross contributions), same-class
masked stats from diag blocks, final loss math in float64.
"""
import numpy as np

# ---- walrus multi-wait workaround ----------------------------------------
"""Patch Bass.to_json_bytes to split multi-wait instructions.

This walrus build (cc-2026-05-04) encodes at most ONE sem-wait per
instruction ("Too many sync wait commands"). Tile's scheduler attaches
several. Splitting extra waits onto standalone EventSemaphore
instructions immediately before the owner, on the same engine, is
semantically identical (engine stream blocks at the same point).
"""
import json


def _split_multi_waits(bir: dict) -> dict:
    for fn in bir.get("functions", []):
        for blk in fn.get("blocks", []):
            newl = []
            for ins in blk.get("instructions", []):
                si = ins.get("sync_info")
                waits = (si or {}).get("on_wait") or []
                if len(waits) > 1:
                    keep = waits[-1]
                    for k, w in enumerate(waits[:-1]):
                        newl.append({
                            "debug": ins.get("debug"),
                            "engine": ins["engine"],
                            "ins": [],
                            "name": f"{ins['name']}-wsplit{k}",
                            "opcode": "EventSemaphore",
                            "outs": [],
                            "sync_info": {"on_update": [], "on_wait": [w]},
                        })
                    si["on_wait"] = [keep]
                newl.append(ins)
            blk["instructions"] = newl
    return bir


def _install_waitsplit():
    import concourse.bass as bass
    if getattr(bass.Bass, "_waitsplit_installed", False):
        return
    orig = bass.Bass.to_json_bytes

    def to_json_bytes(self, *a, **kw):
        raw = orig(self, *a, **kw)
        bir = json.loads(raw)
        bir = _split_multi_waits(bir)
        return json.dumps(bir).encode()

    bass.Bass.to_json_bytes = to_json_bytes
    bass.Bass._waitsplit_installed = True


_install_waitsplit()

# ---- tile exit barrier trim ----------------------------------------
"""Skip the second all-engine barrier in TileContext exit (saves ~2-4us).

The tail sequence is: drain(+waits) -> barrier -> sem clear -> barrier.
The final barrier only guards the PL sem-clear against the *next*
execution's first waits; NRT re-triggers engines only after all streams
halt, so the clear (on PL, before its halt) is already complete.
"""


def _install_barriertrim():
    import concourse.tile as tile
    from concourse.vector_clock import ScopedClock

    if getattr(tile.TileContext, "_barriertrim", False):
        return

    def _drain_and_barrier(self, tick_clock, wait_clock):
        drain_inst = self.nc.sync.drain()
        wait_clock.add_sem_waits(
            drain_inst.ins, ScopedClock({None: tick_clock.global_clock})
        )
        self.nc.all_engine_barrier()
        popped = self.nc._tile_sem_poison_stack.pop()
        assert popped is self._sem_poison
        self.nc.clear_and_free_semaphores(list(self.sems.allocated().values()))

    tile.TileContext._drain_and_barrier = _drain_and_barrier
    tile.TileContext._barriertrim = True


_install_barriertrim()

# ---- kernel ----------------------------------------
"""AHardPair loss kernel v2 — symmetric: each distance-pair computed once.

Row-tile t (128 rows) processes a circular column slab of 33 blocks:
cols [128t, 128t + 4224) mod 8192  (block offsets delta = 0..32).
Pairs with delta in 1..31 are covered once (by the lower tile); the
delta=32 block is covered by BOTH endpoints (host halves it); delta=0
is the diagonal block (poisoned diagonal, same-class handled on host).

Rows are sharded core k <- global row-tiles {k, k+8, ..., k+56}; inputs
are column-rotated per core by 128k so every core runs the identical
program (slot s covers local cols [1024s, 1024s+4224) mod 8192).

Per slot on device:
  G matmuls (bf16) -> psum pieces;  d = sqrt(2 - 2G) (bf16, ACT)
  e1 = exp(40-40d), e2 = exp(22-20d): ACT with free row-sum accum
  colsums of e over cols [128, 4224): ones-matmul -> psum [1,512] pieces
    -> DVE copy -> SBUF -> DRAM  (cross contributions for other rows)
  ship d diag block + delta32 block (bf16) for host-side masked stats.
Host: scatter colsums, delta32 halving, same-class stats, loss in f64.
"""
import concourse.bass as bass
import concourse.tile as tile
from concourse import mybir
from concourse.bass_utils import run_bass_kernel_spmd
from bass_rust import add_dep_helper

N = 8192
D = 128
NC = 8
RPT = 128               # rows per tile/slot
SPC = 8                 # slots per core
SLAB = 33 * RPT         # max slab width (slots 0-3); slots 4-7: 32 blocks
CSW = SLAB - RPT        # max colsum width: 4096


def slab_w(s):
    # slots 0-3 (global tiles < 32) include the delta=32 block; the
    # partner tiles (slots 4-7) get it via the partner's colsum
    return SLAB if s < 4 else 32 * RPT
CHUNK = 512
NCHUNK = N // CHUNK     # 16

F32 = mybir.dt.float32
BF16 = mybir.dt.bfloat16
AF = mybir.ActivationFunctionType
ALU = mybir.AluOpType

ALPHA, BETA = 40.0, 20.0


def _register_const(nc, value, dtype=F32):
    t = nc.alloc_sbuf_tensor(f"const-{dtype.name}-{value}", [128, 1], dtype)
    nc.gpsimd.memset(t.ap(), value)
    nc.const_aps.aps[(dtype, value)] = t.ap()


def build_nc(repeat=1):
    nc = bass.Bass("TRN2", target_bir_lowering=False, debug=False, num_devices=NC)

    xT = nc.dram_tensor("xT", [D, N], BF16, kind="ExternalInput")
    diag14 = nc.dram_tensor("diag14", [RPT, RPT], F32, kind="ExternalInput")
    S1o = nc.dram_tensor("S1", [RPT, SPC], F32, kind="ExternalOutput")
    S2o = nc.dram_tensor("S2", [RPT, SPC], F32, kind="ExternalOutput")
    db0o = nc.dram_tensor("dblk0", [RPT, SPC * RPT], BF16, kind="ExternalOutput")
    CS1o = nc.dram_tensor("CS1", [SPC, CSW], F32, kind="ExternalOutput")
    CS2o = nc.dram_tensor("CS2", [SPC, CSW], F32, kind="ExternalOutput")

    consts = nc.dram_tensor("consts", [RPT, 3], F32, kind="ExternalInput")

    ones_bf16 = nc.const_aps.aps[(BF16, 1.0)]  # [128,1] lhsT for colsums

    import contextlib
    with tile.TileContext(nc) as tc:
        with contextlib.ExitStack() as ctx:
            sing = ctx.enter_context(tc.tile_pool(name="sing", bufs=1))
            dpool = ctx.enter_context(tc.tile_pool(name="dpool", bufs=SPC))
            epool = ctx.enter_context(tc.tile_pool(name="epool", bufs=4))
            cstage = ctx.enter_context(tc.tile_pool(name="cstage", bufs=4))

            xT_ch = [sing.tile([D, CHUNK], BF16, tag=f"xc{j}", name=f"xc{j}") for j in range(NCHUNK)]
            xTn2_t = sing.tile([D, SPC * RPT], BF16)
            dg_t = sing.tile([RPT, RPT], F32)
            s1_t = sing.tile([RPT, SPC], F32)
            s2_t = sing.tile([RPT, SPC], F32)

            consts_t = sing.tile([RPT, 3], F32)
            nc.sync.dma_start(out=consts_t[:, :], in_=consts[:, :])
            b_sqrt = consts_t[:, 0:1]
            b_e1 = consts_t[:, 1:2]
            b_e2 = consts_t[:, 2:3]
            nc.sync.dma_start(out=dg_t[:, :], in_=diag14[:, :])
            # spread the big xT load across two async queues (keep ACT free),
            # in the order slots consume the chunks
            dma_engines = [nc.sync, nc.gpsimd]
            for j in range(NCHUNK):
                eng = dma_engines[j % len(dma_engines)]
                eng.dma_start(
                    out=xT_ch[j][:, :],
                    in_=xT[:, j * CHUNK:(j + 1) * CHUNK],
                )
            # derive lhsT slices (-2 * x rows) on device: slot s rows are
            # cols [0,128) of chunk 2s
            for s in range(SPC):
                nc.vector.tensor_scalar_mul(
                    xTn2_t[:, s * RPT:(s + 1) * RPT],
                    xT_ch[2 * s][:, 0:RPT], -2.0)

            def seg_matmuls(out_ps, s, u0, u1):
                # emit matmuls covering slab-local cols [u0,u1) of slot s.
                # Segments must not cross source 512-chunk boundaries NOR
                # psum bank (512 f32) boundaries of out_ps.
                u = u0
                while u < u1:
                    g = (1024 * s + u) % N          # global-local col
                    gc, go = g // CHUNK, g % CHUNK  # chunk id, offset
                    po = u - u0                     # psum offset
                    seg = min(u1 - u, CHUNK - go, 512 - po % 512)
                    nc.tensor.matmul(
                        out_ps[:, po:po + seg],
                        xTn2_t[:, s * RPT:(s + 1) * RPT],
                        xT_ch[gc][:, go:go + seg],
                        start=True, stop=True,
                    )
                    u += seg

            preload_t = sing.tile([RPT, 1], F32)

            act_chain = []

            def act(*a, **kw):
                inst = nc.scalar.activation(*a, **kw)
                if act_chain:
                    add_dep_helper(inst.ins, act_chain[-1].ins,
                                   reason="act-phase-order")
                act_chain.append(inst)
                return inst

            # preload the sqrt ACT table during the DMA head (needs only
            # the consts tile, which lands first)
            act(preload_t[:, :], consts_t[:, 0:1], AF.Sqrt, bias=b_sqrt,
                scale=1.0)

            for _rep in range(repeat):
                d_tiles = {}
                # ---------- phase 1: matmul + sqrt ----------
                with tc.tile_pool(name="psA", bufs=3, space="PSUM") as psA, \
                     tc.tile_pool(name="psB", bufs=2, space="PSUM") as psB:
                    for s in range(SPC):
                        W = slab_w(s)
                        d_t = dpool.tile([RPT, W], BF16, tag="d")
                        d_tiles[s] = d_t
                        for pi in range(4):
                            ps = psA.tile([RPT, 1024], F32, tag="pA")
                            seg_matmuls(ps, s, pi * 1024, (pi + 1) * 1024)
                            if pi == 0:
                                nc.vector.tensor_tensor(
                                    out=ps[:, 0:RPT], in0=ps[:, 0:RPT],
                                    in1=dg_t[:, :], op=ALU.add,
                                )
                            act(d_t[:, pi * 1024:(pi + 1) * 1024], ps[:, :],
                                AF.Sqrt, bias=b_sqrt, scale=1.0)
                        if W > 4096:
                            psb = psB.tile([RPT, RPT], F32, tag="pB")
                            seg_matmuls(psb, s, 4096, W)
                            act(d_t[:, 4096:W], psb[:, :],
                                AF.Sqrt, bias=b_sqrt, scale=1.0)
                        nc.sync.dma_start(
                            out=db0o[:, s * RPT:(s + 1) * RPT],
                            in_=d_t[:, 0:RPT])

                # ---------- phase 2: exp + rowsum accum + colsums ----------
                with tc.tile_pool(name="psC", bufs=2, space="PSUM") as psC:
                    for s in range(SPC):
                        W = slab_w(s)
                        CW = W - RPT
                        hw = CW // 2           # 2048 or 1984
                        d_t = d_tiles[s]
                        for fi, (sc, bi, acc, cso) in enumerate((
                            (-ALPHA, b_e1, s1_t, CS1o),
                            (-BETA, b_e2, s2_t, CS2o),
                        )):
                            e_t = epool.tile([RPT, W], BF16, tag="e")
                            act(e_t[:, :], d_t[:, :], AF.Exp,
                                bias=bi, scale=sc,
                                accum_out=acc[:, s:s + 1])
                            for h in range(2):
                                csp = psC.tile([1, 2048], F32, tag="cs")
                                done = 0
                                while done < hw:
                                    seg = min(CHUNK, hw - done,
                                              512 - done % 512)
                                    u0 = RPT + h * hw + done
                                    nc.tensor.matmul(
                                        csp[:, done:done + seg],
                                        ones_bf16,
                                        e_t[:, u0:u0 + seg],
                                        start=True, stop=True,
                                    )
                                    done += seg
                                stg = cstage.tile([1, 2048], F32, tag="stg")
                                if s == SPC - 1 and fi == 1:
                                    # tail: ACT is idle after its last exp;
                                    # let it drain these two copies while
                                    # DVE drains the e1 ones
                                    act(stg[:, 0:hw], csp[:, 0:hw], AF.Copy)
                                else:
                                    nc.vector.tensor_copy(stg[:, 0:hw],
                                                          csp[:, 0:hw])
                                nc.sync.dma_start(
                                    out=cso[s:s + 1, h * hw:h * hw + hw],
                                    in_=stg[:, 0:hw])

            nc.sync.dma_start(out=S1o[:, :], in_=s1_t[:, :])
            nc.sync.dma_start(out=S2o[:, :], in_=s2_t[:, :])
    return nc


def make_in_maps(x):
    import ml_dtypes
    dg = (14.0 * np.eye(RPT)).astype(np.float32)
    maps = []
    for c in range(NC):
        xr = np.roll(x, -RPT * c, axis=0)
        maps.append({
            "xT": np.ascontiguousarray(xr.T).astype(ml_dtypes.bfloat16),
            "diag14": dg,
            "consts": np.tile(np.array([[2.0, ALPHA, 22.0]], np.float32), (RPT, 1)),
        })
    return maps


def host_finish(results):
    p = np.arange(RPT)
    M = (((p[:, None] // 8) == (p[None, :] // 8)) &
         (p[:, None] != p[None, :])).astype(np.float64)

    S1 = np.zeros(N); S2 = np.zeros(N)
    same1 = np.zeros(N); same2 = np.zeros(N); B = np.zeros(N)

    for k in range(NC):
        r = results[k]
        s1 = r["S1"].astype(np.float64)
        s2 = r["S2"].astype(np.float64)
        db0 = np.asarray(r["dblk0"], dtype=np.float64)
        cs1 = r["CS1"].astype(np.float64)
        cs2 = r["CS2"].astype(np.float64)
        for s in range(SPC):
            t = k + 8 * s
            rows = 128 * t + p
            d0 = db0[:, s * RPT:(s + 1) * RPT]
            S1[rows] += s1[:, s]
            S2[rows] += s2[:, s]
            e1_0 = np.exp(ALPHA * (1.0 - d0))
            e2_0 = np.exp(BETA * (1.1 - d0))
            e3_0 = np.exp(BETA * (d0 - 0.8))
            same1[rows] += (e1_0 * M).sum(1)
            same2[rows] += (e2_0 * M).sum(1)
            B[rows] += (e3_0 * M).sum(1)
            cw = slab_w(s) - RPT
            cols = (128 * t + RPT + np.arange(cw)) % N
            np.add.at(S1, cols, cs1[s, :cw])
            np.add.at(S2, cols, cs2[s, :cw])

    pos = same1
    neg = S1 - same1
    a_lr = 1.0 - pos / (pos + neg)
    pos_loss = np.log(B)
    neg_loss = np.log(S2 - same2)
    return np.float32(np.mean(a_lr * (pos_loss + neg_loss)))


_NC_CACHE = {}


def run(x, repeat=1):
    key = repeat
    if key not in _NC_CACHE:
        _NC_CACHE[key] = build_nc(repeat=repeat)
    nc = _NC_CACHE[key]
    maps = make_in_maps(x)
    res = run_bass_kernel_spmd(nc, maps, core_ids=list(range(NC)))
    return res.results


def _numpy_reference(x, targets):
    # exact fallback (never expected to trigger): straight port of the
    # reference in float64 on host
    n = x.shape[0]
    sq = (x.astype(np.float64) ** 2).sum(1)
    dist = sq[:, None] + sq[None, :] - 2.0 * (x.astype(np.float64) @ x.T.astype(np.float64))
    dist = np.sqrt(np.clip(dist, 1e-12, None))
    same = targets[:, None] == targets[None, :]
    eye = np.eye(n, dtype=bool)
    pos_mask = same & ~eye
    neg_mask = ~same
    e = np.exp(ALPHA * (1.0 - dist))
    pos_logit = (np.where(pos_mask, e, 0.0)).sum(1)
    neg_logit = (np.where(neg_mask, e, 0.0)).sum(1)
    a_lr = 1.0 - pos_logit / (pos_logit + neg_logit)
    pos_loss = np.log((np.where(pos_mask, np.exp(BETA * (dist - 0.8)), 0.0)).sum(1))
    neg_loss = np.log((np.where(neg_mask, np.exp(BETA * (1.1 - dist)), 0.0)).sum(1))
    return np.float32(np.mean(a_lr * (pos_loss + neg_loss)))


def kernel(inputs, targets):
    x = np.ascontiguousarray(np.asarray(inputs, dtype=np.float32))
    tg = np.asarray(targets)
    # device fast path assumes classes are contiguous 8-blocks (as in
    # setup_inputs); anything else falls back to an exact host compute
    if x.shape != (N, D) or not np.array_equal(
            tg.astype(np.int64), np.arange(N, dtype=np.int64) // 8):
        return _numpy_reference(x, tg)
    return host_finish(run(x, repeat=1))



# revision 25
# speedup vs baseline: 1.3732x; 1.3732x over previous
"""Trainium2 Bass kernel for nn_AHardPair (hard-pair mining loss).

Self-contained: computes loss = mean(a_lr * (pos_loss + neg_loss)) over
8192 L2-normalized embeddings of dim 128, classes = contiguous blocks
of 8 rows.  The 8192x8192 pairwise-distance work is sharded over 8
NeuronCores; each distance pair is computed once (symmetric circular
column slabs; slots 0-3 carry 33 blocks, slots 4-7 carry 32), with
per-core column rotation so all cores run one SPMD program.

v3 engine layout (vs the 3-ACT-pass baseline):
  ACT: d = sqrt(2-2G) (phase A) and e2 = exp(22-20d) w/ rowsum accum
       (phase B) -- two passes per pair element instead of three.
  DVE/GpSimd: e1 = e2*e2 = exp(44-40d) fused with rowsum accum
       (tensor_tensor_reduce / scalar_tensor_tensor).  The e^4 scale
       vs exp(40-40d) cancels in a_lr and is matched on host.
  PE:  distance matmuls + colsums via TRANSPOSED matmuls
       (lhsT = e-chunk [128,128], rhs = ones [128,1] -> psum [128,1]),
       so colsums land partition-major and the [1,2048] DVE staging
       copies of the baseline (70us DVE busy) disappear.
Host: scatter colsums, same-class masked stats from d diag blocks,
final loss math in float64.
"""
import numpy as np

# ---- walrus multi-wait workaround ----------------------------------------
"""Patch Bass.to_json_bytes to split multi-wait instructions.

This walrus build (cc-2026-05-04) encodes at most ONE sem-wait per
instruction ("Too many sync wait commands"). Tile's scheduler attaches
several. Splitting extra waits onto standalone EventSemaphore
instructions immediately before the owner, on the same engine, is
semantically identical (engine stream blocks at the same point).
"""
import json


def _split_multi_waits(bir: dict) -> dict:
    for fn in bir.get("functions", []):
        for blk in fn.get("blocks", []):
            newl = []
            for ins in blk.get("instructions", []):
                si = ins.get("sync_info")
                waits = (si or {}).get("on_wait") or []
                if len(waits) > 1:
                    keep = waits[-1]
                    for k, w in enumerate(waits[:-1]):
                        newl.append({
                            "debug": ins.get("debug"),
                            "engine": ins["engine"],
                            "ins": [],
                            "name": f"{ins['name']}-wsplit{k}",
                            "opcode": "EventSemaphore",
                            "outs": [],
                            "sync_info": {"on_update": [], "on_wait": [w]},
                        })
                    si["on_wait"] = [keep]
                newl.append(ins)
            blk["instructions"] = newl
    return bir


def _install_waitsplit():
    import concourse.bass as bass
    if getattr(bass.Bass, "_waitsplit_installed", False):
        return
    orig = bass.Bass.to_json_bytes

    def to_json_bytes(self, *a, **kw):
        raw = orig(self, *a, **kw)
        bir = json.loads(raw)
        bir = _split_multi_waits(bir)
        return json.dumps(bir).encode()

    bass.Bass.to_json_bytes = to_json_bytes
    bass.Bass._waitsplit_installed = True


_install_waitsplit()

# ---- tile exit barrier trim ----------------------------------------
"""Skip the second all-engine barrier in TileContext exit (saves ~2-4us).

The tail sequence is: drain(+waits) -> barrier -> sem clear -> barrier.
The final barrier only guards the PL sem-clear against the *next*
execution's first waits; NRT re-triggers engines only after all streams
halt, so the clear (on PL, before its halt) is already complete.
"""


def _install_barriertrim():
    import concourse.tile as tile
    from concourse.vector_clock import ScopedClock

    if getattr(tile.TileContext, "_barriertrim", False):
        return

    def _drain_and_barrier(self, tick_clock, wait_clock):
        drain_inst = self.nc.sync.drain()
        wait_clock.add_sem_waits(
            drain_inst.ins, ScopedClock({None: tick_clock.global_clock})
        )
        self.nc.all_engine_barrier()
        popped = self.nc._tile_sem_poison_stack.pop()
        assert popped is self._sem_poison
        self.nc.clear_and_free_semaphores(list(self.sems.allocated().values()))

    tile.TileContext._drain_and_barrier = _drain_and_barrier
    tile.TileContext._barriertrim = True


_install_barriertrim()

# ---- kernel ----------------------------------------
import concourse.bass as bass
import concourse.tile as tile
from concourse import mybir
from concourse.bass_utils import run_bass_kernel_spmd
from bass_rust import add_dep_helper

N = 8192
D = 128
NC = 8
RPT = 128               # rows per tile/slot
SPC = 8                 # slots per core
SLAB = 33 * RPT         # max slab width (slots 0-3); slots 4-7: 32 blocks
CHUNK = 512
NCHUNK = N // CHUNK     # 16

F32 = mybir.dt.float32
BF16 = mybir.dt.bfloat16
AF = mybir.ActivationFunctionType
ALU = mybir.AluOpType

ALPHA, BETA = 40.0, 20.0

# which slots run the e1 square+rowsum on gpsimd instead of DVE.
# Empty: this walrus build rejects TensorScalarPtr on the Pool engine.
GP_SLOTS = ()


def slab_w(s):
    # slots 0-3 (global tiles < 32) include the delta=32 block
    return SLAB if s < 4 else 32 * RPT


def build_nc(repeat=1):
    nc = bass.Bass("TRN2", target_bir_lowering=False, debug=False, num_devices=NC)

    xT = nc.dram_tensor("xT", [D, N], BF16, kind="ExternalInput")
    # 14*I poison added to the diag psum block before sqrt (keeps the
    # scalar-engine sqrt input strictly positive; diag-block stats are
    # rebuilt on host from DB anyway)
    diagP = nc.dram_tensor("diagP", [RPT, RPT], F32, kind="ExternalInput")
    consts = nc.dram_tensor("consts", [RPT, 2], F32, kind="ExternalInput")

    # single consolidated f32 output:
    # [CS1 s0-6 (224) | CS2 s0-6 (224) | CS1 s7 (32) | CS2 s7 (32) |
    #  S1 (9) | S2 (9)]  -- slot-7/sums at the end so the bulk can ship
    # before the drain
    OUTW = SPC * 32 * 2 + 2 * (SPC + 1)
    EARLYW = (SPC - 1) * 32 * 2   # 448
    OUTo = nc.dram_tensor("OUT", [RPT, OUTW], F32, kind="ExternalOutput")
    DBo = nc.dram_tensor("DB", [RPT, SPC * RPT], BF16, kind="ExternalOutput")

    ones_bf16 = nc.const_aps.aps[(BF16, 1.0)]  # [128,1]

    import contextlib
    with tile.TileContext(nc) as tc:
        with contextlib.ExitStack() as ctx:
            sing = ctx.enter_context(tc.tile_pool(name="sing", bufs=1))
            dpool = ctx.enter_context(tc.tile_pool(name="dpool", bufs=SPC))
            e2pool = ctx.enter_context(tc.tile_pool(name="e2pool", bufs=3))
            e1pool = ctx.enter_context(tc.tile_pool(name="e1pool", bufs=3))

            xT_ch = [sing.tile([D, CHUNK], BF16, tag=f"xc{j}", name=f"xc{j}")
                     for j in range(NCHUNK)]
            xTn2_t = sing.tile([D, SPC * RPT], BF16)
            dg_t = sing.tile([RPT, RPT], F32)
            out_t = sing.tile([RPT, OUTW], F32)
            # slots 4-7 stage only 31 of their 32 colsum cols; zero the
            # whole tile once so the consolidated DMAs read defined data
            nc.gpsimd.memset(out_t[:, :], 0.0)

            def st1_ap(s, nch):
                # CS1: slots 0-6 at 32*s; slot 7 parked after the early blk
                base = 32 * s if s < SPC - 1 else EARLYW
                return out_t[:, base:base + nch]

            def st2_ap(s, nch):
                base = (SPC - 1) * 32 + 32 * s if s < SPC - 1 else EARLYW + 32
                return out_t[:, base:base + nch]

            s1_t = out_t[:, EARLYW + 64:EARLYW + 64 + SPC + 1]
            s2_t = out_t[:, EARLYW + 64 + SPC + 1:OUTW]

            consts_t = sing.tile([RPT, 2], F32)
            nc.sync.dma_start(out=consts_t[:, :], in_=consts[:, :])
            b_sqrt = consts_t[:, 0:1]
            b_e2 = consts_t[:, 1:2]
            nc.gpsimd.dma_start(out=dg_t[:, :], in_=diagP[:, :])
            # spread the big xT load across two async queues, in the order
            # slots consume the chunks
            dma_engines = [nc.sync, nc.gpsimd]
            for j in range(NCHUNK):
                eng = dma_engines[j % len(dma_engines)]
                eng.dma_start(
                    out=xT_ch[j][:, :],
                    in_=xT[:, j * CHUNK:(j + 1) * CHUNK],
                )
            # derive lhsT slices (-2 * x rows) on device: slot s rows are
            # cols [0,128) of chunk 2s
            for s in range(SPC):
                nc.vector.tensor_scalar_mul(
                    xTn2_t[:, s * RPT:(s + 1) * RPT],
                    xT_ch[2 * s][:, 0:RPT], -2.0)

            def seg_matmuls(out_ps, s, u0, u1):
                # emit matmuls covering slab-local cols [u0,u1) of slot s.
                # Segments must not cross source 512-chunk boundaries NOR
                # psum bank (512 f32) boundaries of out_ps.
                u = u0
                while u < u1:
                    g = (1024 * s + u) % N          # global-local col
                    gc, go = g // CHUNK, g % CHUNK  # chunk id, offset
                    po = u - u0                     # psum offset
                    seg = min(u1 - u, CHUNK - go, 512 - po % 512)
                    nc.tensor.matmul(
                        out_ps[:, po:po + seg],
                        xTn2_t[:, s * RPT:(s + 1) * RPT],
                        xT_ch[gc][:, go:go + seg],
                        start=True, stop=True,
                    )
                    u += seg

            preload_t = sing.tile([RPT, 1], F32)

            act_chain = []

            def act(*a, **kw):
                inst = nc.scalar.activation(*a, **kw)
                if act_chain:
                    add_dep_helper(inst.ins, act_chain[-1].ins,
                                   reason="act-phase-order")
                act_chain.append(inst)
                return inst

            # preload the sqrt ACT table during the DMA head (needs only
            # the consts tile, which lands first)
            act(preload_t[:, :], consts_t[:, 0:1], AF.Sqrt, bias=b_sqrt,
                scale=1.0)

            for _rep in range(repeat):
                d_tiles = {}
                # ---------- phase A: matmul + sqrt ----------
                # psum pieces of 1024 f32 (2 banks); 4 bufs fill all 8
                # banks and give PE enough lookahead to hide slot
                # boundaries.  The 128-wide tail piece of slots 0-3
                # borrows a pool slot but only uses its first 128 cols.
                with tc.tile_pool(name="psA", bufs=4, space="PSUM") as psA:
                    for s in range(SPC):
                        W = slab_w(s)
                        d_t = dpool.tile([RPT, W], BF16, tag="d")
                        d_tiles[s] = d_t
                        pieces = [(u, u + 1024) for u in range(0, 4096, 1024)]
                        if W > 4096:
                            pieces.append((4096, W))
                        for (u0, u1) in pieces:
                            ps = psA.tile([RPT, 1024], F32, tag="pA")
                            seg_matmuls(ps, s, u0, u1)
                            if u0 == 0:
                                nc.vector.tensor_tensor(
                                    out=ps[:, 0:RPT], in0=ps[:, 0:RPT],
                                    in1=dg_t[:, :], op=ALU.add,
                                )
                            act(d_t[:, u0:u1], ps[:, 0:u1 - u0],
                                AF.Sqrt, bias=b_sqrt, scale=1.0)
                            if u0 == 0:
                                # d diag block ships via the gpsimd queue
                                # (SP's DGE seq is the busier one)
                                nc.gpsimd.dma_start(
                                    out=DBo[:, s * RPT:(s + 1) * RPT],
                                    in_=d_t[:, 0:RPT])

                # ---------- phase B: exp + squares + transposed colsums ----
                with tc.tile_pool(name="psC", bufs=6, space="PSUM") as psC:
                    e1_tiles = {}
                    cs_tiles = {}

                    def colsums(e_t, base, c0, c1, csp=None):
                        # transposed colsum matmuls over slab chunks
                        # [c0,c1); e_t's col 0 is slab col `base`.
                        # Each csp gets a FULL psum bank (512 f32): psum
                        # dependencies are tracked at bank granularity,
                        # so sharing a bank serializes unrelated slots.
                        if csp is None:
                            csp = psC.tile([RPT, 512], F32, tag="cs")
                        for c in range(c0, c1):
                            lo = RPT + c * RPT - base
                            nc.tensor.matmul(
                                csp[:, c:c + 1],
                                e_t[:, lo:lo + RPT],
                                ones_bf16,
                                start=True, stop=True,
                            )
                        return csp

                    def square(e1_ap, e2_ap, acc, on_gp):
                        # e1 = e2*e2 (= exp(44-40d)) with free rowsum accum
                        # (scalar_tensor_tensor: out = (in0*1) * in1; the
                        # ISA-level tensor_tensor_reduce fails codegen on
                        # this walrus build)
                        eng = nc.gpsimd if on_gp else nc.vector
                        eng.scalar_tensor_tensor(
                            out=e1_ap, in0=e2_ap, scalar=1.0,
                            in1=e2_ap, op0=ALU.mult, op1=ALU.mult,
                            accum_out=acc)

                    def stage(s, nch, on_act=False):
                        # copy both colsum psums for slot s into the
                        # consolidated output tile.  DVE normally; the
                        # drain-tail copies ride the then-idle ACT (Copy
                        # shares the exp table set -> no table reload).
                        csp2 = cs_tiles[(s, 2)]
                        csp1 = cs_tiles[(s, 1)]
                        if on_act:
                            act(st2_ap(s, nch), csp2[:, 0:nch], AF.Copy)
                            act(st1_ap(s, nch), csp1[:, 0:nch], AF.Copy)
                        else:
                            nc.vector.tensor_copy(st2_ap(s, nch),
                                                  csp2[:, 0:nch])
                            nc.vector.tensor_copy(st1_ap(s, nch),
                                                  csp1[:, 0:nch])

                    LAST = SPC - 1
                    for s in range(SPC):
                        # phase B skips the diag block: slab cols [128, W)
                        W = slab_w(s)
                        CW = W - RPT
                        nch = CW // RPT
                        pnch = (slab_w(s - 1) - RPT) // RPT
                        d_t = d_tiles[s]
                        if s < LAST:
                            e2_t = e2pool.tile([RPT, CW], BF16, tag="e2")
                            e1_t = e1pool.tile([RPT, CW], BF16, tag="e1")
                            e1_tiles[s] = (e1_t,)
                            act(e2_t[:, :], d_t[:, RPT:W], AF.Exp,
                                bias=b_e2, scale=-BETA,
                                accum_out=s2_t[:, s:s + 1])
                            square(e1_t[:, :], e2_t[:, :],
                                   s1_t[:, s:s + 1], s in GP_SLOTS)
                            cs_tiles[(s, 2)] = colsums(e2_t, RPT, 0, nch)
                            if s > 0:
                                cs_tiles[(s - 1, 1)] = colsums(
                                    e1_tiles[s - 1][0], RPT, 0, pnch)
                                stage(s - 1, pnch)
                        else:
                            # last slot runs as two asymmetric pieces
                            # (cols [128,3072) + [3072,4096)) so the
                            # DVE/PE drain after the final short exp is
                            # minimal
                            h = 3 * 1024
                            ch = (h - 2 * RPT) // RPT + 1  # chunks in A: 23
                            e2a = e2pool.tile([RPT, h - RPT], BF16, tag="e2")
                            e1a = e1pool.tile([RPT, h - RPT], BF16, tag="e1")
                            act(e2a[:, :], d_t[:, RPT:h], AF.Exp,
                                bias=b_e2, scale=-BETA,
                                accum_out=s2_t[:, s:s + 1])
                            square(e1a[:, :], e2a[:, :],
                                   s1_t[:, s:s + 1], False)
                            csp2 = colsums(e2a, RPT, 0, ch)
                            cs_tiles[(s, 2)] = csp2
                            cs_tiles[(s - 1, 1)] = colsums(
                                e1_tiles[s - 1][0], RPT, 0, pnch)
                            stage(s - 1, pnch)
                            # bulk of the results can ship while the last
                            # piece is still in flight
                            nc.sync.dma_start(out=OUTo[:, 0:EARLYW],
                                              in_=out_t[:, 0:EARLYW])
                            e2b = e2pool.tile([RPT, W - h], BF16, tag="e2")
                            e1b = e1pool.tile([RPT, W - h], BF16, tag="e1")
                            act(e2b[:, :], d_t[:, h:W], AF.Exp,
                                bias=b_e2, scale=-BETA,
                                accum_out=s2_t[:, s + 1:s + 2])
                            square(e1b[:, :], e2b[:, :],
                                   s1_t[:, s + 1:s + 2], False)
                            colsums(e2b, h, ch, nch, csp2)
                            csp1 = colsums(e1a, RPT, 0, ch)
                            cs_tiles[(s, 1)] = csp1
                            colsums(e1b, h, ch, nch, csp1)
                            stage(s, nch, on_act=True)

            # tail DMA: slot-7 colsums + row sums
            nc.sync.dma_start(out=OUTo[:, EARLYW:OUTW],
                              in_=out_t[:, EARLYW:OUTW])
    return nc


def make_in_maps(x):
    import ml_dtypes
    maps = []
    for c in range(NC):
        xr = np.roll(x, -RPT * c, axis=0)
        maps.append({
            "xT": np.ascontiguousarray(xr.T).astype(ml_dtypes.bfloat16),
            "diagP": (14.0 * np.eye(RPT)).astype(np.float32),
            "consts": np.tile(np.array([[2.0, 22.0]], np.float32), (RPT, 1)),
        })
    return maps


def host_finish(results):
    p = np.arange(RPT)
    M = (((p[:, None] // 8) == (p[None, :] // 8)) &
         (p[:, None] != p[None, :])).astype(np.float64)

    S1 = np.zeros(N); S2 = np.zeros(N)
    same1 = np.zeros(N); same2 = np.zeros(N); B = np.zeros(N)
    S1b = S1.reshape(N // RPT, RPT)
    S2b = S2.reshape(N // RPT, RPT)

    EARLYW = (SPC - 1) * 32 * 2
    for k in range(NC):
        r = results[k]
        out = r["OUT"].astype(np.float64)
        # layout: CS1 s0-6 | CS2 s0-6 | CS1 s7 | CS2 s7 | S1 | S2
        cs1 = np.concatenate(
            [out[:, 0:(SPC - 1) * 32], out[:, EARLYW:EARLYW + 32]], axis=1)
        cs2 = np.concatenate(
            [out[:, (SPC - 1) * 32:EARLYW], out[:, EARLYW + 32:EARLYW + 64]],
            axis=1)
        s1 = out[:, EARLYW + 64:EARLYW + 64 + SPC + 1]
        s2 = out[:, EARLYW + 64 + SPC + 1:]
        db = np.asarray(r["DB"], dtype=np.float64)
        for s in range(SPC):
            t = k + 8 * s
            rows = 128 * t + p
            # diag-block distances; exact diagonal is garbage/NaN on
            # device (unpoisoned sqrt) -- overwrite before exponentials
            d0 = db[:, s * RPT:(s + 1) * RPT].copy()
            d0[p, p] = 10.0
            d0 = np.nan_to_num(d0, nan=10.0)
            S1[rows] += s1[:, s]
            S2[rows] += s2[:, s]
            if s == SPC - 1:
                # last slot's second half accumulates into the extra col
                S1[rows] += s1[:, SPC]
                S2[rows] += s2[:, SPC]
            # device e1 convention is exp(44-40d) = e2^2; the e^4 factor
            # vs exp(40-40d) cancels in a_lr as long as host matches it
            e1_0 = np.exp(44.0 - ALPHA * d0)
            e2_0 = np.exp(22.0 - BETA * d0)
            e3_0 = np.exp(BETA * d0 - 16.0)
            same1[rows] += (e1_0 * M).sum(1)
            same2[rows] += (e2_0 * M).sum(1)
            B[rows] += (e3_0 * M).sum(1)
            nch = (slab_w(s) - RPT) // RPT
            # colsum chunk c covers global row-block (t+1+c) mod 64,
            # row-in-block p  ->  CS[p, 32s+c]
            for c in range(nch):
                blk = (t + 1 + c) % (N // RPT)
                S1b[blk] += cs1[:, 32 * s + c]
                S2b[blk] += cs2[:, 32 * s + c]

    # device sums exclude the diag block entirely, so S1/S2 are pure
    # negative-pair sums already
    pos = same1
    neg = S1
    a_lr = 1.0 - pos / (pos + neg)
    pos_loss = np.log(B)
    neg_loss = np.log(S2)
    return np.float32(np.mean(a_lr * (pos_loss + neg_loss)))


_NC_CACHE = {}


def run(x, repeat=1):
    key = repeat
    if key not in _NC_CACHE:
        _NC_CACHE[key] = build_nc(repeat=repeat)
    nc = _NC_CACHE[key]
    maps = make_in_maps(x)
    res = run_bass_kernel_spmd(nc, maps, core_ids=list(range(NC)))
    return res.results


def _numpy_reference(x, targets):
    # exact fallback (never expected to trigger): straight port of the
    # reference in float64 on host
    n = x.shape[0]
    sq = (x.astype(np.float64) ** 2).sum(1)
    dist = sq[:, None] + sq[None, :] - 2.0 * (x.astype(np.float64) @ x.T.astype(np.float64))
    dist = np.sqrt(np.clip(dist, 1e-12, None))
    same = targets[:, None] == targets[None, :]
    eye = np.eye(n, dtype=bool)
    pos_mask = same & ~eye
    neg_mask = ~same
    e = np.exp(ALPHA * (1.0 - dist))
    pos_logit = (np.where(pos_mask, e, 0.0)).sum(1)
    neg_logit = (np.where(neg_mask, e, 0.0)).sum(1)
    a_lr = 1.0 - pos_logit / (pos_logit + neg_logit)
    pos_loss = np.log((np.where(pos_mask, np.exp(BETA * (dist - 0.8)), 0.0)).sum(1))
    neg_loss = np.log((np.where(neg_mask, np.exp(BETA * (1.1 - dist)), 0.0)).sum(1))
    return np.float32(np.mean(a_lr * (pos_loss + neg_loss)))


def kernel(inputs, targets):
    x = np.ascontiguousarray(np.asarray(inputs, dtype=np.float32))
    tg = np.asarray(targets)
    # device fast path assumes classes are contiguous 8-blocks (as in
    # setup_inputs); anything else falls back to an exact host compute
    if x.shape != (N, D) or not np.array_equal(
            tg.astype(np.int64), np.arange(N, dtype=np.int64) // 8):
        return _numpy_reference(x, tg)
    return host_finish(run(x, repeat=1))
